# revision 16
# baseline (speedup 1.0000x reference)
"""Multi-Head Latent Attention (MLA) TRN2 Bass kernel, 8-core parallel.

Sharding: batch x heads. Cores 0-3 own batch 0, cores 4-7 batch 1; within a
batch group each core owns 4 heads (tensor-parallel on q/kv_up/o_proj).
The host precomputes W_eff = W_kv_down @ W_kv_up (fp32), so each core runs
kv = x @ W_eff[:, its heads] directly -- no replicated latent projection.
Each core writes ONE bf16 o_proj partial [S, D]; the host sums 4 per batch.

All heavy matmuls run bf16 (same PE rate as fp32r, half the SBUF/DMA).
Dataflow is fully transposed (no device transposes except kvT -> kv_nat):
  xT [D, S] (host transpose, bf16) ->
  qT  = Wq^T  xT   [4*Dh, S]   (per-head tiles, stays in SBUF)
  kvT = Weff^T xT  [4*Dh, S]   (bf16, scores-lhsT)
  kvn[h] = PE-transpose of kvT[h] per 128-tile -> [S, Dh] bf16 (out-mm lhsT)
  scoresT[keys, q] = kvT-tile^T qT-slice (bf16); e = exp(scores*scale) bf16
  outT[dh, q] += kvn-tile^T e  (accumulate over key tiles in PSUM)
  denom: GpSimd sums the first 6 e tiles, DVE the last 10 (DVE is ~2x
  faster, so its backlog drains right after the last exp), both into fp32
  accs (mixed bf16+f32 tensor_add, validated on HW); acc_d+acc_g combined
  on DVE into an f32r tile (a real rounding cast -- the BIR verifier
  rejects bitcasts feeding f32r matmuls); ones-matmul replicates the
  128-partition sum; reciprocal_approx_fast; outT = ps_o * rcp (bf16).
  The finalize is deferred into the next block's filler pops so the PE
  never stalls waiting for the accumulators.
  o_proj: final[s, D] = sum_h outT[h]-tile^T Wo[h] (bf16), evict bf16, DMA.
Softmax max-subtraction is skipped: |scores*scale| < ~1.3.

Projection/transpose/o_proj chains are queued and drained inside the
attention kt-loops so the PE never idles while ACT runs exp. PSUM: scores
2x2 banks ("ss", never shared with fillers), attn-out 2 ("po"),
proj/transpose/o_proj 1-bank chunks share "x1" (2 banks).
Measured 394-395us on HW (baseline 460us), rel err 4.2e-3 vs the 2e-2
gate. Rejected by experiment: fp8/e4m3 anywhere in the data path (2.3-4e-2
error -- attention output is a weighted mean, so relative error passes
through unsuppressed), DoublePixel perf mode (silent no-op for bf16),
mixed f32r x bf16 matmuls (BIR verifier), weights-before-xT DMA order and
an o_proj head-pair split (both measured slower).
"""
import sys

sys.path.insert(0, "/opt/trn_rl_repo")

import numpy as np  # noqa: E402
import ml_dtypes  # noqa: E402

B = 2
S = 2048
D = 2048
H = 16
DH = 128
P = 128
N_CORES = 8
H_LOC = 4                     # heads per core
HW = H_LOC * DH               # 512
D_T = D // P                  # 16
KT = S // P                   # 16 key tiles
QW = 1024                     # q block width
NQB = S // QW                 # 2 q blocks per head
SC = 512                      # psum bank slice (max matmul moving width)
SCALE = float(1.0 / np.sqrt(DH))
N_GPS = 6                     # denominator kt-tiles accumulated on GpSimd


def _build_nc():
    import concourse.tile as tile
    import concourse.mybir as mybir
    from concourse import bacc

    f32 = mybir.dt.float32
    f32r = mybir.dt.float32r
    bf16 = mybir.dt.bfloat16
    EXP = mybir.ActivationFunctionType.Exp

    nc = bacc.Bacc("TRN2", target_bir_lowering=False, debug=False)

    xT = nc.dram_tensor("xT", [D, S], bf16, kind="ExternalInput").ap()
    wq = nc.dram_tensor("wq", [D, HW], bf16, kind="ExternalInput").ap()
    we = nc.dram_tensor("we", [D, HW], bf16, kind="ExternalInput").ap()
    wo = nc.dram_tensor("wo", [HW, D], bf16, kind="ExternalInput").ap()
    ident = nc.dram_tensor("ident", [P, P], bf16, kind="ExternalInput").ap()
    ones_d = nc.dram_tensor("ones", [P, P], f32r, kind="ExternalInput").ap()
    out = nc.dram_tensor("out", [S, D], bf16, kind="ExternalOutput").ap()

    with tile.TileContext(nc) as tc:
        with tc.tile_pool(name="w", bufs=1) as wp, \
             tc.tile_pool(name="big", bufs=1) as bigp, \
             tc.tile_pool(name="sm", bufs=1) as smp, \
             tc.tile_pool(name="ps", bufs=1, space="PSUM") as psp:

            # ---- input DMAs (three queues) ----
            # xT round-robin over 4 queues (~2MB each) so the 8MB load hits
            # the BW floor instead of serializing 6MB behind one queue; the
            # weights follow ON THE SAME queues so the first projection
            # chain still starts dense (all xt resident when wq lands --
            # a trickle-start paced by DMA measured slower)
            qs = [nc.sync, nc.gpsimd, nc.scalar]
            xt_t = []
            for dt_i in range(D_T):
                t = bigp.tile([P, S], bf16, tag=f"xt{dt_i}", name=f"xt{dt_i}")
                qs[dt_i % 3].dma_start(t[:], xT[dt_i * P:(dt_i + 1) * P, :])
                xt_t.append(t)
            wq_t, we_t = [], []
            for dt_i in range(D_T):
                t = wp.tile([P, HW], bf16, tag=f"wq{dt_i}", name=f"wq{dt_i}")
                (nc.sync if dt_i % 2 == 0 else nc.scalar).dma_start(
                    t[:], wq[dt_i * P:(dt_i + 1) * P, :])
                wq_t.append(t)
                t = wp.tile([P, HW], bf16, tag=f"we{dt_i}", name=f"we{dt_i}")
                nc.gpsimd.dma_start(t[:], we[dt_i * P:(dt_i + 1) * P, :])
                we_t.append(t)
            ident_t = wp.tile([P, P], bf16, tag="ident", name="ident")
            nc.sync.dma_start(ident_t[:], ident[:, :])
            ones_t = wp.tile([P, P], f32r, tag="ones", name="ones")
            nc.sync.dma_start(ones_t[:], ones_d[:, :])
            wo_t = []
            for ht in range(H_LOC):
                t = wp.tile([P, D], bf16, tag=f"wo{ht}", name=f"wo{ht}")
                nc.gpsimd.dma_start(t[:], wo[ht * P:(ht + 1) * P, :])
                wo_t.append(t)

            # ---- persistent SBUF tensors ----
            qT = [bigp.tile([P, S], bf16, tag=f"qT{h}", name=f"qT{h}")
                  for h in range(H_LOC)]
            kvT = [bigp.tile([P, S], bf16, tag=f"kvT{h}", name=f"kvT{h}")
                   for h in range(H_LOC)]
            kvn = [bigp.tile([P, S], bf16, tag=f"kvn{h}", name=f"kvn{h}")
                   for h in range(H_LOC)]
            outT = [bigp.tile([P, S], bf16, tag=f"outT{h}",
                              name=f"outT{h}")
                    for h in range(H_LOC)]

            pending = []

            def _proj_chain(h, kind, c):
                # c indexes a 512-wide s-chunk; psum on the 1-bank "x1" tag
                # so the scores "ss" ring keeps clean double-buffering
                wt = wq_t if kind == "q" else we_t
                dst = qT[h] if kind == "q" else kvT[h]
                ps = psp.tile([P, SC], f32, tag="x1", bufs=2,
                              name=f"pp_{kind}{h}_{c}")
                for dt_i in range(D_T):
                    nc.tensor.matmul(
                        ps[:],
                        wt[dt_i][:, h * P:(h + 1) * P],
                        xt_t[dt_i][:, c * SC:(c + 1) * SC],
                        start=(dt_i == 0), stop=(dt_i == D_T - 1))
                nc.scalar.copy(dst[:, c * SC:(c + 1) * SC], ps[:])

            def _transp_chain(h, half):
                ps = psp.tile([P, 8 * P], bf16, tag="x1", bufs=2,
                              name=f"tp_{h}_{half}")
                for j in range(8):
                    kt_i = half * 8 + j
                    nc.tensor.transpose(ps[:, j * P:(j + 1) * P],
                                        kvT[h][:, kt_i * P:(kt_i + 1) * P],
                                        ident_t[:])
                nc.vector.tensor_copy(
                    kvn[h][:, half * 8 * P:(half + 1) * 8 * P], ps[:])

            def _oproj_chain(st, dq):
                pc = psp.tile([P, SC], f32, tag="x1", bufs=2,
                              name=f"pc_{st}_{dq}")
                for ht in range(H_LOC):
                    nc.tensor.matmul(pc[:, :SC],
                                     outT[ht][:, st * P:(st + 1) * P],
                                     wo_t[ht][:, dq * SC:(dq + 1) * SC],
                                     start=(ht == 0), stop=(ht == H_LOC - 1))
                # wq weight tiles are dead by o_proj time; reuse their space
                fin = wp.tile([P, SC], bf16, tag=f"wq{(st * 4 + dq) % 6}",
                              name=f"fin_{st}_{dq}")
                nc.vector.tensor_copy(fin[:], pc[:, :SC])
                nc.sync.dma_start(out[st * P:(st + 1) * P,
                                      dq * SC:(dq + 1) * SC], fin[:])

            def _attention(h, qb):
                acc_d = smp.tile([P, QW], f32, tag="accd", name=f"ad{h}{qb}")
                acc_g = smp.tile([P, QW], f32, tag="accg", name=f"ag{h}{qb}")
                ps_o = psp.tile([P, QW], f32, tag="po", bufs=1,
                                name=f"po{h}{qb}")
                es = [None] * KT

                def _consume(kt):
                    e = es[kt]
                    for i in range(2):
                        nc.tensor.matmul(ps_o[:, i * SC:(i + 1) * SC],
                                         kvn[h][:, kt * P:(kt + 1) * P],
                                         e[:, i * SC:(i + 1) * SC],
                                         start=(kt == 0), stop=(kt == KT - 1))

                for kt in range(KT):
                    pss = psp.tile([P, QW], f32, tag="ss", bufs=2,
                                   name=f"pss{h}{qb}{kt}")
                    for i in range(2):
                        nc.tensor.matmul(
                            pss[:, i * SC:(i + 1) * SC],
                            kvT[h][:, kt * P:(kt + 1) * P],
                            qT[h][:, qb * QW + i * SC:qb * QW + (i + 1) * SC],
                            start=True, stop=True)
                    e = smp.tile([P, QW], bf16, tag="e", bufs=3,
                                 name=f"e{h}{qb}{kt}")
                    nc.scalar.activation(e[:], pss[:], EXP, scale=SCALE)
                    es[kt] = e
                    ef = e[:]
                    # gpsimd (slower) gets the EARLY tiles so its backlog
                    # drains long before the block finalize needs acc_g
                    if kt < N_GPS:
                        if kt == 0:
                            nc.gpsimd.tensor_copy(acc_g[:], ef)
                        else:
                            nc.gpsimd.tensor_add(acc_g[:], acc_g[:], ef)
                    else:
                        if kt == N_GPS:
                            nc.vector.tensor_copy(acc_d[:], ef)
                        else:
                            nc.vector.tensor_add(acc_d[:], acc_d[:], ef)
                    if kt >= 1:
                        _consume(kt - 1)
                    if pending and (kt % 3 == 2 or
                                    (kt % 3 == 1 and len(pending) > 16)):
                        pending.pop(0)()
                _consume(KT - 1)

                def _finalize(h=h, qb=qb, acc_d=acc_d, acc_g=acc_g,
                              ps_o=ps_o):
                    acc_r = smp.tile([P, QW], f32r, tag="accr",
                                     name=f"ar{h}{qb}")
                    nc.vector.tensor_add(acc_r[:], acc_d[:], acc_g[:])
                    psd = psp.tile([P, QW], f32, tag="ss", bufs=2,
                                   name=f"psd{h}{qb}")
                    for i in range(2):
                        isl = slice(i * SC, (i + 1) * SC)
                        nc.tensor.matmul(psd[:, isl], ones_t[:],
                                         acc_r[:, isl],
                                         start=True, stop=True)
                    rcp = smp.tile([P, QW], f32, tag="rcp", name=f"rcp{h}{qb}")
                    nc.vector.reciprocal_approx_fast(out=rcp[:], in_=psd[:])
                    nc.vector.tensor_mul(outT[h][:, qb * QW:(qb + 1) * QW],
                                         ps_o[:], rcp[:])
                # defer: runs as a filler inside the NEXT block, by which
                # time both denominator accumulators have drained
                pending.insert(0, _finalize)

            # ---- schedule ----
            # proj(0) + transp(0) up front; everything else queued as filler
            for kind in ("q", "kv"):
                for c in range(4):
                    _proj_chain(0, kind, c)
            for half in range(2):
                _transp_chain(0, half)
            for h in range(1, H_LOC):
                for kind in ("q", "kv"):
                    for c in range(4):
                        pending.append(
                            lambda h=h, kind=kind, c=c:
                            _proj_chain(h, kind, c))
                for half in range(2):
                    pending.append(
                        lambda h=h, half=half: _transp_chain(h, half))

            for h in range(H_LOC):
                for qb in range(NQB):
                    _attention(h, qb)
                    if h == H_LOC - 1:
                        for st in range(qb * 8, (qb + 1) * 8):
                            for dq in range(D // SC):
                                pending.append(
                                    lambda st=st, dq=dq:
                                    _oproj_chain(st, dq))
            for ch in pending:
                ch()

    nc.compile()
    return nc


_NC_CACHE = None


def _get_nc():
    global _NC_CACHE
    if _NC_CACHE is None:
        _NC_CACHE = _build_nc()
    return _NC_CACHE


def _bf16(a):
    return np.ascontiguousarray(a, dtype=np.float32).astype(ml_dtypes.bfloat16)


def _run(x, W_q, W_kv_down, W_kv_up, W_o, trace=False):
    from concourse.bass_utils import run_bass_kernel_spmd

    x = np.asarray(x, dtype=np.float32)
    W_q = np.asarray(W_q, dtype=np.float32)
    W_eff = np.asarray(W_kv_down, dtype=np.float32) @ \
        np.asarray(W_kv_up, dtype=np.float32)
    W_o = np.asarray(W_o, dtype=np.float32)

    nc = _get_nc()

    ident = np.eye(P, dtype=np.float32)
    ones = np.ones((P, P), np.float32)
    xT_b = [_bf16(x[b].T) for b in range(B)]

    in_maps = []
    for c in range(N_CORES):
        bc = c // 4
        hs = slice((c % 4) * HW, (c % 4 + 1) * HW)
        in_maps.append({
            "xT": xT_b[bc],
            "wq": _bf16(W_q[:, hs]),
            "we": _bf16(W_eff[:, hs]),
            "wo": _bf16(W_o[hs, :]),
            "ident": _bf16(ident),
            "ones": ones,
        })

    r = run_bass_kernel_spmd(nc, in_maps, list(range(N_CORES)), trace=trace)
    outs = []
    for bc in range(B):
        acc = None
        for i in range(4):
            part = r.results[4 * bc + i]["out"].astype(np.float32)
            acc = part if acc is None else acc + part
        outs.append(acc)
    return np.stack(outs).astype(np.float32), r


def kernel(x, W_q, W_kv_down, W_kv_up, W_o):
    out, _ = _run(x, W_q, W_kv_down, W_kv_up, W_o, trace=False)
    return out


# revision 17
# speedup vs baseline: 1.0174x; 1.0174x over previous
"""Multi-Head Latent Attention (MLA) TRN2 Bass kernel, 8-core parallel.

Sharding: batch x heads. Cores 0-3 own batch 0, cores 4-7 batch 1; within a
batch group each core owns 4 heads (tensor-parallel on q/kv_up/o_proj).
The host precomputes W_eff = W_kv_down @ W_kv_up (fp32), so each core runs
kv = x @ W_eff[:, its heads] directly -- no replicated latent projection.
Each core writes ONE bf16 o_proj partial [S, D]; the host sums 4 per batch.

All heavy matmuls run bf16 (same PE rate as fp32r, half the SBUF/DMA).
Dataflow is fully transposed (no device transposes except kvT -> kv_nat):
  xT [D, S] (host transpose, bf16) ->
  qT  = Wq^T  xT   [4*Dh, S]   (per-head tiles, stays in SBUF)
  kvT = Weff^T xT  [4*Dh, S]   (bf16, scores-lhsT)
  kvn[h] = PE-transpose of kvT[h] per 128-tile -> [S, Dh] bf16 (out-mm lhsT)
  scoresT[keys, q] = kvT-tile^T qT-slice (bf16); e = exp(scores*scale) bf16
  outT[dh, q] += kvn-tile^T e  (accumulate over key tiles in PSUM)
  denom: GpSimd sums the first 6 e tiles, DVE the last 10 (DVE is ~2x
  faster, so its backlog drains right after the last exp), both into fp32
  accs (mixed bf16+f32 tensor_add, validated on HW); acc_d+acc_g combined
  on DVE into an f32r tile (a real rounding cast -- the BIR verifier
  rejects bitcasts feeding f32r matmuls); ones-matmul replicates the
  128-partition sum; reciprocal_approx_fast; outT = ps_o * rcp (bf16).
  The finalize is deferred into the next block's filler pops so the PE
  never stalls waiting for the accumulators.
  o_proj: final[s, D] = sum_h outT[h]-tile^T Wo[h] (bf16), evict bf16, DMA.
Softmax max-subtraction is skipped: |scores*scale| < ~1.3.

Projection/transpose/o_proj chains are queued and drained inside the
attention kt-loops so the PE never idles while ACT runs exp. PSUM: scores
2x2 banks ("ss", never shared with fillers), attn-out 2 ("po"),
proj/transpose/o_proj 1-bank chunks share "x1" (2 banks).
Measured 394-395us on HW (baseline 460us), rel err 4.2e-3 vs the 2e-2
gate. Rejected by experiment: fp8/e4m3 anywhere in the data path (2.3-4e-2
error -- attention output is a weighted mean, so relative error passes
through unsuppressed), DoublePixel perf mode (silent no-op for bf16),
mixed f32r x bf16 matmuls (BIR verifier), weights-before-xT DMA order and
an o_proj head-pair split (both measured slower).
"""
import sys

sys.path.insert(0, "/opt/trn_rl_repo")

import numpy as np  # noqa: E402
import ml_dtypes  # noqa: E402

B = 2
S = 2048
D = 2048
H = 16
DH = 128
P = 128
N_CORES = 8
H_LOC = 4                     # heads per core
HW = H_LOC * DH               # 512
D_T = D // P                  # 16
KT = S // P                   # 16 key tiles
QW = 1024                     # q block width
NQB = S // QW                 # 2 q blocks per head
SC = 512                      # psum bank slice (max matmul moving width)
SCALE = float(1.0 / np.sqrt(DH))
N_GPS = 6                     # denominator kt-tiles accumulated on GpSimd


def _build_nc():
    import concourse.tile as tile
    import concourse.mybir as mybir
    from concourse import bacc

    f32 = mybir.dt.float32
    f32r = mybir.dt.float32r
    bf16 = mybir.dt.bfloat16
    EXP = mybir.ActivationFunctionType.Exp

    nc = bacc.Bacc("TRN2", target_bir_lowering=False, debug=False)

    xT = nc.dram_tensor("xT", [D, S], bf16, kind="ExternalInput").ap()
    wq = nc.dram_tensor("wq", [D, HW], bf16, kind="ExternalInput").ap()
    we = nc.dram_tensor("we", [D, HW], bf16, kind="ExternalInput").ap()
    wo = nc.dram_tensor("wo", [HW, D], bf16, kind="ExternalInput").ap()
    ident = nc.dram_tensor("ident", [P, P], bf16, kind="ExternalInput").ap()
    ones_d = nc.dram_tensor("ones", [P, P], f32r, kind="ExternalInput").ap()
    out = nc.dram_tensor("out", [S, D], bf16, kind="ExternalOutput").ap()

    with tile.TileContext(nc) as tc:
        with tc.tile_pool(name="w", bufs=1) as wp, \
             tc.tile_pool(name="big", bufs=1) as bigp, \
             tc.tile_pool(name="sm", bufs=1) as smp, \
             tc.tile_pool(name="ps", bufs=1, space="PSUM") as psp:

            # ---- input DMAs (two queues) ----
            # NOTE: measured alternatives that LOST: weights-before-xT
            # (trickle-start paced by DMA, PE never ramps) and a 3-queue
            # xT split (startup is HBM-BW-capped, not queue-capped)
            xt_t = []
            for dt_i in range(D_T):
                t = bigp.tile([P, S], bf16, tag=f"xt{dt_i}", name=f"xt{dt_i}")
                eng = nc.sync if dt_i % 2 == 0 else nc.gpsimd
                eng.dma_start(t[:], xT[dt_i * P:(dt_i + 1) * P, :])
                xt_t.append(t)
            wq_t, we_t = [], []
            for dt_i in range(D_T):
                t = wp.tile([P, HW], bf16, tag=f"wq{dt_i}", name=f"wq{dt_i}")
                nc.sync.dma_start(t[:], wq[dt_i * P:(dt_i + 1) * P, :])
                wq_t.append(t)
                t = wp.tile([P, HW], bf16, tag=f"we{dt_i}", name=f"we{dt_i}")
                nc.gpsimd.dma_start(t[:], we[dt_i * P:(dt_i + 1) * P, :])
                we_t.append(t)
            ident_t = wp.tile([P, P], bf16, tag="ident", name="ident")
            nc.sync.dma_start(ident_t[:], ident[:, :])
            ones_t = wp.tile([P, P], f32r, tag="ones", name="ones")
            nc.sync.dma_start(ones_t[:], ones_d[:, :])
            wo_t = []
            for ht in range(H_LOC):
                t = wp.tile([P, D], bf16, tag=f"wo{ht}", name=f"wo{ht}")
                nc.gpsimd.dma_start(t[:], wo[ht * P:(ht + 1) * P, :])
                wo_t.append(t)

            # ---- persistent SBUF tensors ----
            qT = [bigp.tile([P, S], bf16, tag=f"qT{h}", name=f"qT{h}")
                  for h in range(H_LOC)]
            kvT = [bigp.tile([P, S], bf16, tag=f"kvT{h}", name=f"kvT{h}")
                   for h in range(H_LOC)]
            kvn = [bigp.tile([P, S], bf16, tag=f"kvn{h}", name=f"kvn{h}")
                   for h in range(H_LOC)]
            outT = [bigp.tile([P, S], bf16, tag=f"outT{h}",
                              name=f"outT{h}")
                    for h in range(H_LOC)]

            pending = []

            def _proj_chain(h, kind, c):
                # c indexes a 512-wide s-chunk; psum on the 1-bank "x1" tag
                # so the scores "ss" ring keeps clean double-buffering
                wt = wq_t if kind == "q" else we_t
                dst = qT[h] if kind == "q" else kvT[h]
                ps = psp.tile([P, SC], f32, tag="x1", bufs=2,
                              name=f"pp_{kind}{h}_{c}")
                for dt_i in range(D_T):
                    nc.tensor.matmul(
                        ps[:],
                        wt[dt_i][:, h * P:(h + 1) * P],
                        xt_t[dt_i][:, c * SC:(c + 1) * SC],
                        start=(dt_i == 0), stop=(dt_i == D_T - 1))
                nc.scalar.copy(dst[:, c * SC:(c + 1) * SC], ps[:])

            def _transp_chain(h, half):
                ps = psp.tile([P, 8 * P], bf16, tag="x1", bufs=2,
                              name=f"tp_{h}_{half}")
                for j in range(8):
                    kt_i = half * 8 + j
                    nc.tensor.transpose(ps[:, j * P:(j + 1) * P],
                                        kvT[h][:, kt_i * P:(kt_i + 1) * P],
                                        ident_t[:])
                nc.vector.tensor_copy(
                    kvn[h][:, half * 8 * P:(half + 1) * 8 * P], ps[:])

            def _oproj_chain(st, dq):
                pc = psp.tile([P, SC], f32, tag="x1", bufs=2,
                              name=f"pc_{st}_{dq}")
                for ht in range(H_LOC):
                    nc.tensor.matmul(pc[:, :SC],
                                     outT[ht][:, st * P:(st + 1) * P],
                                     wo_t[ht][:, dq * SC:(dq + 1) * SC],
                                     start=(ht == 0), stop=(ht == H_LOC - 1))
                # wq weight tiles are dead by o_proj time; reuse their space
                fin = wp.tile([P, SC], bf16, tag=f"wq{(st * 4 + dq) % 6}",
                              name=f"fin_{st}_{dq}")
                nc.vector.tensor_copy(fin[:], pc[:, :SC])
                nc.sync.dma_start(out[st * P:(st + 1) * P,
                                      dq * SC:(dq + 1) * SC], fin[:])

            def _attention(h, qb):
                acc_d = smp.tile([P, QW], f32, tag="accd", name=f"ad{h}{qb}")
                acc_g = smp.tile([P, QW], f32, tag="accg", name=f"ag{h}{qb}")
                ps_o = psp.tile([P, QW], f32, tag="po", bufs=1,
                                name=f"po{h}{qb}")
                es = [None] * KT

                def _consume(kt):
                    e = es[kt]
                    for i in range(2):
                        nc.tensor.matmul(ps_o[:, i * SC:(i + 1) * SC],
                                         kvn[h][:, kt * P:(kt + 1) * P],
                                         e[:, i * SC:(i + 1) * SC],
                                         start=(kt == 0), stop=(kt == KT - 1))

                for kt in range(KT):
                    pss = psp.tile([P, QW], f32, tag="ss", bufs=2,
                                   name=f"pss{h}{qb}{kt}")
                    for i in range(2):
                        nc.tensor.matmul(
                            pss[:, i * SC:(i + 1) * SC],
                            kvT[h][:, kt * P:(kt + 1) * P],
                            qT[h][:, qb * QW + i * SC:qb * QW + (i + 1) * SC],
                            start=True, stop=True)
                    e = smp.tile([P, QW], bf16, tag="e", bufs=3,
                                 name=f"e{h}{qb}{kt}")
                    nc.scalar.activation(e[:], pss[:], EXP, scale=SCALE)
                    es[kt] = e
                    ef = e[:]
                    # gpsimd (slower) gets the EARLY tiles so its backlog
                    # drains long before the block finalize needs acc_g
                    if kt < N_GPS:
                        if kt == 0:
                            nc.gpsimd.tensor_copy(acc_g[:], ef)
                        else:
                            nc.gpsimd.tensor_add(acc_g[:], acc_g[:], ef)
                    else:
                        if kt == N_GPS:
                            nc.vector.tensor_copy(acc_d[:], ef)
                        else:
                            nc.vector.tensor_add(acc_d[:], acc_d[:], ef)
                    if kt >= 1:
                        _consume(kt - 1)
                    if pending and (kt % 3 == 2 or
                                    (kt % 3 == 1 and len(pending) > 16)):
                        pending.pop(0)()
                _consume(KT - 1)

                def _finalize(h=h, qb=qb, acc_d=acc_d, acc_g=acc_g,
                              ps_o=ps_o):
                    acc_r = smp.tile([P, QW], f32r, tag="accr",
                                     name=f"ar{h}{qb}")
                    nc.vector.tensor_add(acc_r[:], acc_d[:], acc_g[:])
                    psd = psp.tile([P, QW], f32, tag="ss", bufs=2,
                                   name=f"psd{h}{qb}")
                    for i in range(2):
                        isl = slice(i * SC, (i + 1) * SC)
                        nc.tensor.matmul(psd[:, isl], ones_t[:],
                                         acc_r[:, isl],
                                         start=True, stop=True)
                    rcp = smp.tile([P, QW], f32, tag="rcp", name=f"rcp{h}{qb}")
                    nc.vector.reciprocal_approx_fast(out=rcp[:], in_=psd[:])
                    nc.vector.tensor_mul(outT[h][:, qb * QW:(qb + 1) * QW],
                                         ps_o[:], rcp[:])
                # defer: runs as a filler inside the NEXT block, by which
                # time both denominator accumulators have drained
                pending.insert(0, _finalize)

            # ---- schedule ----
            # proj(0) + transp(0) up front; everything else queued as filler
            for kind in ("q", "kv"):
                for c in range(4):
                    _proj_chain(0, kind, c)
            for half in range(2):
                _transp_chain(0, half)
            for h in range(1, H_LOC):
                for kind in ("q", "kv"):
                    for c in range(4):
                        pending.append(
                            lambda h=h, kind=kind, c=c:
                            _proj_chain(h, kind, c))
                for half in range(2):
                    pending.append(
                        lambda h=h, half=half: _transp_chain(h, half))

            for h in range(H_LOC):
                for qb in range(NQB):
                    _attention(h, qb)
                    if h == H_LOC - 1:
                        for st in range(qb * 8, (qb + 1) * 8):
                            for dq in range(D // SC):
                                pending.append(
                                    lambda st=st, dq=dq:
                                    _oproj_chain(st, dq))
            for ch in pending:
                ch()

    nc.compile()
    return nc


_NC_CACHE = None


def _get_nc():
    global _NC_CACHE
    if _NC_CACHE is None:
        _NC_CACHE = _build_nc()
    return _NC_CACHE


def _bf16(a):
    return np.ascontiguousarray(a, dtype=np.float32).astype(ml_dtypes.bfloat16)


def _run(x, W_q, W_kv_down, W_kv_up, W_o, trace=False):
    from concourse.bass_utils import run_bass_kernel_spmd

    x = np.asarray(x, dtype=np.float32)
    W_q = np.asarray(W_q, dtype=np.float32)
    W_eff = np.asarray(W_kv_down, dtype=np.float32) @ \
        np.asarray(W_kv_up, dtype=np.float32)
    W_o = np.asarray(W_o, dtype=np.float32)

    nc = _get_nc()

    ident = np.eye(P, dtype=np.float32)
    ones = np.ones((P, P), np.float32)
    xT_b = [_bf16(x[b].T) for b in range(B)]

    in_maps = []
    for c in range(N_CORES):
        bc = c // 4
        hs = slice((c % 4) * HW, (c % 4 + 1) * HW)
        in_maps.append({
            "xT": xT_b[bc],
            "wq": _bf16(W_q[:, hs]),
            "we": _bf16(W_eff[:, hs]),
            "wo": _bf16(W_o[hs, :]),
            "ident": _bf16(ident),
            "ones": ones,
        })

    r = run_bass_kernel_spmd(nc, in_maps, list(range(N_CORES)), trace=trace)
    outs = []
    for bc in range(B):
        acc = None
        for i in range(4):
            part = r.results[4 * bc + i]["out"].astype(np.float32)
            acc = part if acc is None else acc + part
        outs.append(acc)
    return np.stack(outs).astype(np.float32), r


def kernel(x, W_q, W_kv_down, W_kv_up, W_o):
    out, _ = _run(x, W_q, W_kv_down, W_kv_up, W_o, trace=False)
    return out


# revision 18
# speedup vs baseline: 1.0319x; 1.0143x over previous
"""Multi-Head Latent Attention (MLA) TRN2 Bass kernel, 8-core parallel.

Sharding: batch x heads. Cores 0-3 own batch 0, cores 4-7 batch 1; within a
batch group each core owns 4 heads (tensor-parallel on q/kv_up/o_proj).
The host precomputes W_eff = W_kv_down @ W_kv_up (fp32), so each core runs
kv = x @ W_eff[:, its heads] directly -- no replicated latent projection.
Each core writes ONE bf16 o_proj partial [S, D]; the host sums 4 per batch.

All heavy matmuls run bf16 (same PE rate as fp32r, half the SBUF/DMA).
Dataflow is fully transposed (no device transposes except kvT -> kv_nat):
  xT [D, S] (host transpose, bf16) ->
  qT  = Wq^T  xT   [4*Dh, S]   (per-head tiles, stays in SBUF)
  kvT = Weff^T xT  [4*Dh, S]   (bf16, scores-lhsT)
  kvn[h] = PE-transpose of kvT[h] per 128-tile -> [S, Dh] bf16 (out-mm lhsT)
  scoresT[keys, q] = kvT-tile^T qT-slice (bf16); e = exp(scores*scale) bf16
  outT[dh, q] += kvn-tile^T e  (accumulate over key tiles in PSUM)
  denom: GpSimd sums the first 6 e tiles, DVE the last 10 (DVE is ~2x
  faster, so its backlog drains right after the last exp), both into fp32
  accs (mixed bf16+f32 tensor_add, validated on HW); acc_d+acc_g combined
  on DVE into an f32r tile (a real rounding cast -- the BIR verifier
  rejects bitcasts feeding f32r matmuls); ones-matmul replicates the
  128-partition sum; reciprocal_approx_fast; outT = ps_o * rcp (bf16).
  The finalize is deferred into the next block's filler pops so the PE
  never stalls waiting for the accumulators.
  o_proj: final[s, D] = sum_h outT[h]-tile^T Wo[h] (bf16), evict bf16, DMA.
Softmax max-subtraction is skipped: |scores*scale| < ~1.3.

Projection/transpose/o_proj chains are queued and drained inside the
attention kt-loops so the PE never idles while ACT runs exp. PSUM: scores
2x2 banks ("ss", never shared with fillers), attn-out 2 ("po"),
proj/transpose/o_proj 1-bank chunks share "x1" (2 banks).
Measured 394-395us on HW (baseline 460us), rel err 4.2e-3 vs the 2e-2
gate. Rejected by experiment: fp8/e4m3 anywhere in the data path (2.3-4e-2
error -- attention output is a weighted mean, so relative error passes
through unsuppressed), DoublePixel perf mode (silent no-op for bf16),
mixed f32r x bf16 matmuls (BIR verifier), weights-before-xT DMA order and
an o_proj head-pair split (both measured slower).
"""
import sys

sys.path.insert(0, "/opt/trn_rl_repo")

import numpy as np  # noqa: E402
import ml_dtypes  # noqa: E402

B = 2
S = 2048
D = 2048
H = 16
DH = 128
P = 128
N_CORES = 8
H_LOC = 4                     # heads per core
HW = H_LOC * DH               # 512
D_T = D // P                  # 16
KT = S // P                   # 16 key tiles
QW = 1024                     # q block width
NQB = S // QW                 # 2 q blocks per head
SC = 512                      # psum bank slice (max matmul moving width)
SCALE = float(1.0 / np.sqrt(DH))
N_GPS = 6                     # denominator kt-tiles accumulated on GpSimd


def _build_nc():
    import concourse.tile as tile
    import concourse.mybir as mybir
    from concourse import bacc

    f32 = mybir.dt.float32
    f32r = mybir.dt.float32r
    bf16 = mybir.dt.bfloat16
    EXP = mybir.ActivationFunctionType.Exp

    nc = bacc.Bacc("TRN2", target_bir_lowering=False, debug=False)

    xT = nc.dram_tensor("xT", [D, S], bf16, kind="ExternalInput").ap()
    wq = nc.dram_tensor("wq", [D, HW], bf16, kind="ExternalInput").ap()
    we = nc.dram_tensor("we", [D, HW], bf16, kind="ExternalInput").ap()
    wo = nc.dram_tensor("wo", [HW, D], bf16, kind="ExternalInput").ap()
    ident = nc.dram_tensor("ident", [P, P], bf16, kind="ExternalInput").ap()
    ones_d = nc.dram_tensor("ones", [P, P], f32r, kind="ExternalInput").ap()
    out = nc.dram_tensor("out", [S, D], bf16, kind="ExternalOutput").ap()

    with tile.TileContext(nc) as tc:
        with tc.tile_pool(name="w", bufs=1) as wp, \
             tc.tile_pool(name="big", bufs=1) as bigp, \
             tc.tile_pool(name="sm", bufs=1) as smp, \
             tc.tile_pool(name="ps", bufs=1, space="PSUM") as psp:

            # ---- input DMAs (two queues) ----
            # NOTE: measured alternatives that LOST: weights-before-xT
            # (trickle-start paced by DMA, PE never ramps) and a 3-queue
            # xT split (startup is HBM-BW-capped, not queue-capped)
            xt_t = []
            for dt_i in range(D_T):
                t = bigp.tile([P, S], bf16, tag=f"xt{dt_i}", name=f"xt{dt_i}")
                eng = nc.sync if dt_i % 2 == 0 else nc.gpsimd
                eng.dma_start(t[:], xT[dt_i * P:(dt_i + 1) * P, :])
                xt_t.append(t)
            wq_t, we_t = [], []
            for dt_i in range(D_T):
                t = wp.tile([P, HW], bf16, tag=f"wq{dt_i}", name=f"wq{dt_i}")
                nc.sync.dma_start(t[:], wq[dt_i * P:(dt_i + 1) * P, :])
                wq_t.append(t)
                t = wp.tile([P, HW], bf16, tag=f"we{dt_i}", name=f"we{dt_i}")
                nc.gpsimd.dma_start(t[:], we[dt_i * P:(dt_i + 1) * P, :])
                we_t.append(t)
            ident_t = wp.tile([P, P], bf16, tag="ident", name="ident")
            nc.sync.dma_start(ident_t[:], ident[:, :])
            ones_t = wp.tile([P, P], f32r, tag="ones", name="ones")
            nc.sync.dma_start(ones_t[:], ones_d[:, :])
            wo_t = []
            for ht in range(H_LOC):
                t = wp.tile([P, D], bf16, tag=f"wo{ht}", name=f"wo{ht}")
                nc.gpsimd.dma_start(t[:], wo[ht * P:(ht + 1) * P, :])
                wo_t.append(t)

            # ---- persistent SBUF tensors ----
            qT = [bigp.tile([P, S], bf16, tag=f"qT{h}", name=f"qT{h}")
                  for h in range(H_LOC)]
            kvT = [bigp.tile([P, S], bf16, tag=f"kvT{h}", name=f"kvT{h}")
                   for h in range(H_LOC)]
            kvn = [bigp.tile([P, S], bf16, tag=f"kvn{h}", name=f"kvn{h}")
                   for h in range(H_LOC)]
            outT = [bigp.tile([P, S], bf16, tag=f"outT{h}",
                              name=f"outT{h}")
                    for h in range(H_LOC)]

            pending = []

            def _proj_chain(h, kind, c):
                # c indexes a 512-wide s-chunk; psum on the 1-bank "x1" tag
                # so the scores "ss" ring keeps clean double-buffering
                wt = wq_t if kind == "q" else we_t
                dst = qT[h] if kind == "q" else kvT[h]
                ps = psp.tile([P, SC], f32, tag="x1", bufs=2,
                              name=f"pp_{kind}{h}_{c}")
                for dt_i in range(D_T):
                    nc.tensor.matmul(
                        ps[:],
                        wt[dt_i][:, h * P:(h + 1) * P],
                        xt_t[dt_i][:, c * SC:(c + 1) * SC],
                        start=(dt_i == 0), stop=(dt_i == D_T - 1))
                nc.scalar.copy(dst[:, c * SC:(c + 1) * SC], ps[:])

            def _transp_chain(h, half):
                ps = psp.tile([P, 8 * P], bf16, tag="x1", bufs=2,
                              name=f"tp_{h}_{half}")
                for j in range(8):
                    kt_i = half * 8 + j
                    nc.tensor.transpose(ps[:, j * P:(j + 1) * P],
                                        kvT[h][:, kt_i * P:(kt_i + 1) * P],
                                        ident_t[:])
                nc.vector.tensor_copy(
                    kvn[h][:, half * 8 * P:(half + 1) * 8 * P], ps[:])

            def _oproj_chain(st, dq):
                pc = psp.tile([P, SC], f32, tag="x1", bufs=2,
                              name=f"pc_{st}_{dq}")
                for ht in range(H_LOC):
                    nc.tensor.matmul(pc[:, :SC],
                                     outT[ht][:, st * P:(st + 1) * P],
                                     wo_t[ht][:, dq * SC:(dq + 1) * SC],
                                     start=(ht == 0), stop=(ht == H_LOC - 1))
                # wq weight tiles are dead by o_proj time; reuse their space
                fin = wp.tile([P, SC], bf16, tag=f"wq{(st * 4 + dq) % 6}",
                              name=f"fin_{st}_{dq}")
                nc.vector.tensor_copy(fin[:], pc[:, :SC])
                nc.sync.dma_start(out[st * P:(st + 1) * P,
                                      dq * SC:(dq + 1) * SC], fin[:])

            def _attention(h, qb):
                acc_d = smp.tile([P, QW], f32, tag="accd", name=f"ad{h}{qb}")
                acc_g = smp.tile([P, QW], f32, tag="accg", name=f"ag{h}{qb}")
                ps_o = psp.tile([P, QW], f32, tag="po", bufs=1,
                                name=f"po{h}{qb}")
                es = [None] * KT

                def _consume(kt):
                    e = es[kt]
                    for i in range(2):
                        nc.tensor.matmul(ps_o[:, i * SC:(i + 1) * SC],
                                         kvn[h][:, kt * P:(kt + 1) * P],
                                         e[:, i * SC:(i + 1) * SC],
                                         start=(kt == 0), stop=(kt == KT - 1))

                for kt in range(KT):
                    pss = psp.tile([P, QW], f32, tag="ss", bufs=2,
                                   name=f"pss{h}{qb}{kt}")
                    for i in range(2):
                        nc.tensor.matmul(
                            pss[:, i * SC:(i + 1) * SC],
                            kvT[h][:, kt * P:(kt + 1) * P],
                            qT[h][:, qb * QW + i * SC:qb * QW + (i + 1) * SC],
                            start=True, stop=True)
                    e = smp.tile([P, QW], bf16, tag="e", bufs=4,
                                 name=f"e{h}{qb}{kt}")
                    nc.scalar.activation(e[:], pss[:], EXP, scale=SCALE)
                    es[kt] = e
                    ef = e[:]
                    # gpsimd (slower) gets the EARLY tiles so its backlog
                    # drains long before the block finalize needs acc_g
                    if kt < N_GPS:
                        if kt == 0:
                            nc.gpsimd.tensor_copy(acc_g[:], ef)
                        else:
                            nc.gpsimd.tensor_add(acc_g[:], acc_g[:], ef)
                    else:
                        if kt == N_GPS:
                            nc.vector.tensor_copy(acc_d[:], ef)
                        else:
                            nc.vector.tensor_add(acc_d[:], acc_d[:], ef)
                    # pop BEFORE consume: the deferred finalize's psd
                    # matmuls then issue ahead of the ps_o-WAR-blocked
                    # consume(0) at the top of each block
                    if pending and (kt % 3 == 1 or
                                    (kt % 3 == 2 and len(pending) > 16)):
                        pending.pop(0)()
                    if kt >= 1:
                        _consume(kt - 1)
                _consume(KT - 1)

                def _finalize(h=h, qb=qb, acc_d=acc_d, acc_g=acc_g,
                              ps_o=ps_o):
                    acc_r = smp.tile([P, QW], f32r, tag="accr",
                                     name=f"ar{h}{qb}")
                    nc.vector.tensor_add(acc_r[:], acc_d[:], acc_g[:])
                    psd = psp.tile([P, QW], f32, tag="ss", bufs=2,
                                   name=f"psd{h}{qb}")
                    for i in range(2):
                        isl = slice(i * SC, (i + 1) * SC)
                        nc.tensor.matmul(psd[:, isl], ones_t[:],
                                         acc_r[:, isl],
                                         start=True, stop=True)
                    rcp = smp.tile([P, QW], f32, tag="rcp", name=f"rcp{h}{qb}")
                    nc.vector.reciprocal_approx_fast(out=rcp[:], in_=psd[:])
                    nc.vector.tensor_mul(outT[h][:, qb * QW:(qb + 1) * QW],
                                         ps_o[:], rcp[:])
                # defer: runs as a filler inside the NEXT block, by which
                # time both denominator accumulators have drained
                pending.insert(0, _finalize)

            # ---- schedule ----
            # proj(0) + transp(0) up front; everything else queued as filler
            for kind in ("q", "kv"):
                for c in range(4):
                    _proj_chain(0, kind, c)
            for half in range(2):
                _transp_chain(0, half)
            for h in range(1, H_LOC):
                for kind in ("q", "kv"):
                    for c in range(4):
                        pending.append(
                            lambda h=h, kind=kind, c=c:
                            _proj_chain(h, kind, c))
                for half in range(2):
                    pending.append(
                        lambda h=h, half=half: _transp_chain(h, half))

            for h in range(H_LOC):
                for qb in range(NQB):
                    _attention(h, qb)
                    if h == H_LOC - 1:
                        for st in range(qb * 8, (qb + 1) * 8):
                            for dq in range(D // SC):
                                pending.append(
                                    lambda st=st, dq=dq:
                                    _oproj_chain(st, dq))
            for ch in pending:
                ch()

    nc.compile()
    return nc


_NC_CACHE = None


def _get_nc():
    global _NC_CACHE
    if _NC_CACHE is None:
        _NC_CACHE = _build_nc()
    return _NC_CACHE


def _bf16(a):
    return np.ascontiguousarray(a, dtype=np.float32).astype(ml_dtypes.bfloat16)


def _run(x, W_q, W_kv_down, W_kv_up, W_o, trace=False):
    from concourse.bass_utils import run_bass_kernel_spmd

    x = np.asarray(x, dtype=np.float32)
    W_q = np.asarray(W_q, dtype=np.float32)
    W_eff = np.asarray(W_kv_down, dtype=np.float32) @ \
        np.asarray(W_kv_up, dtype=np.float32)
    W_o = np.asarray(W_o, dtype=np.float32)

    nc = _get_nc()

    ident = np.eye(P, dtype=np.float32)
    ones = np.ones((P, P), np.float32)
    xT_b = [_bf16(x[b].T) for b in range(B)]

    in_maps = []
    for c in range(N_CORES):
        bc = c // 4
        hs = slice((c % 4) * HW, (c % 4 + 1) * HW)
        in_maps.append({
            "xT": xT_b[bc],
            "wq": _bf16(W_q[:, hs]),
            "we": _bf16(W_eff[:, hs]),
            "wo": _bf16(W_o[hs, :]),
            "ident": _bf16(ident),
            "ones": ones,
        })

    r = run_bass_kernel_spmd(nc, in_maps, list(range(N_CORES)), trace=trace)
    outs = []
    for bc in range(B):
        acc = None
        for i in range(4):
            part = r.results[4 * bc + i]["out"].astype(np.float32)
            acc = part if acc is None else acc + part
        outs.append(acc)
    return np.stack(outs).astype(np.float32), r


def kernel(x, W_q, W_kv_down, W_kv_up, W_o):
    out, _ = _run(x, W_q, W_kv_down, W_kv_up, W_o, trace=False)
    return out
